# revision 1
# baseline (speedup 1.0000x reference)
"""Banded (sliding-window) multi-head attention on 8 Trainium2 NeuronCores.

Problem: B=2, S=2048, D=512, H=8 heads (hd=64), window=256 (|i-j| <= 128),
  qkv = x @ Wqkv + bqkv           -> per-head q,k,v
  scores = (q k^T masked to band) / 8 ; softmax ; out = (attn v) @ Wo + bo

Sharding: core = (batch b in {0,1}) x (head-group g in {0..3}); each core
computes 2 heads over the full sequence of one batch element plus the o_proj
partial product for its heads' embed slice. The host sums the 4 partials per
batch; bo/4 is added on each of the 4 cores so the partial sums carry the
full bias.

Device-side layout notes (all matmuls in float32r = full-rate fp32):
  - qkv projection is computed TRANSPOSED: qkvT[fo, s] with the 384 output
    features permuted to [q0|q1|k0|k1|v0|v1] (64 cols each), so Q^T/K^T/V^T
    per head live at partition offsets {0,64} directly.
  - scores are computed key-major per 128-key block kb against the 1-3
    query blocks within the band window: ST[key, query] in PSUM, then
    ACT: P = exp(ST/8 + kmask[key]) and DVE: P *= trimask (band edges).
  - V^T is re-transposed to natural [key, hd] via the PE, augmented with a
    ones column so the attention-weight row sums (softmax denominators)
    drop out of the same AV matmul (row 64 of the [65, q] output).
  - AV accumulates per 512-query chunk over <=6 key blocks into one PSUM
    bank using the per-element has_written semantics (first matmul
    start=True clears the bank; later ones overwrite-or-accumulate).
  - o_proj: outT[fo, s] = Wo_g^T valsT (+ bo/4), DMA'd out transposed;
    host re-transposes and sums the 4 partials per batch.

Performance (concourse cost model, per core): 54.16us makespan; engine busy
ACT 34.2 / PE 30.4 / DVE 28.2 / DMA 27.1us, all three compute engines at
85-100% through the 20-45us core. Measured accuracy vs the fp32 reference:
3.2e-4 max relative error. The remaining makespan over the busiest engine
is (a) the input-bandwidth ramp (x/weight streaming paces qkv, 0-15us),
(b) the ACT-exclusive exp stream (19.6us + per-op drains; no other engine
has transcendentals), (c) the HBM-write tail for the final 1MB output
chunk plus the fixed Tile exit barrier (~4.3us). Engine/ring/pool
assignments below are each the measured optimum of their neighborhood
(~80 A/B variants); see the session memory for the negative-result
catalog before re-trying alternatives.
"""

import numpy as np

import concourse.bass as bass  # noqa: F401  (engine types via nc)
import concourse.mybir as mybir
import concourse.tile as tile
from concourse import bacc
from concourse.bass_utils import run_bass_kernel_spmd

B, S, DIN, E = 2, 2048, 512, 512
H, HD = 8, 64
NB = S // 128      # 16 key/query blocks of 128
NCHUNK = S // 512  # 4 query chunks of 512
F32 = mybir.dt.float32
F32R = mybir.dt.float32r
EXPF = mybir.ActivationFunctionType.Exp

_CACHE = {}
_PB = [2, 2, 2, 2]  # psum pool bufs: qkv, misc, st, ot
LAST_RESULTS = None  # BassKernelResults of the most recent run (for test.py)


def _build_nc():
    nc = bacc.Bacc(None, target_bir_lowering=False, debug=False)

    xt = nc.dram_tensor("xt", [4, DIN, 512], F32R, kind="ExternalInput")
    wq = nc.dram_tensor("wq", [128, 4, 384], F32R, kind="ExternalInput")
    wo = nc.dram_tensor("wo", [128, E], F32R, kind="ExternalInput")
    km = nc.dram_tensor("km", [128, NB], F32, kind="ExternalInput")
    tm = nc.dram_tensor("tm", [128, 384], F32, kind="ExternalInput")
    bo4 = nc.dram_tensor("bo4", [128, 4], F32, kind="ExternalInput")
    idin = nc.dram_tensor("idin", [128, 128], F32R, kind="ExternalInput")
    outt = nc.dram_tensor("outt", [E, S], F32, kind="ExternalOutput")

    with tile.TileContext(nc) as tc:
        with (
            tc.tile_pool(name="sb", bufs=1) as sb,
            tc.tile_pool(name="ps_qkv", bufs=_PB[0], space="PSUM") as ps_qkv,
            tc.tile_pool(name="ps_misc", bufs=_PB[1], space="PSUM") as ps_misc,
            tc.tile_pool(name="ps_st", bufs=_PB[2], space="PSUM") as ps_st,
            tc.tile_pool(name="ps_ot", bufs=_PB[3], space="PSUM") as ps_ot,
            tc.tile_pool(name="small", bufs=4) as small,
        ):
            xt_sb = sb.tile([128, 4, 4, 512], F32R)   # [p, kchunk, qchunk, q]
            wq_sb = sb.tile([128, 4, 384], F32R)      # [p, kchunk, fo]
            wo_sb = sb.tile([128, E], F32R)
            km_sb = sb.tile([128, NB], F32)
            tm2_sb = sb.tile([128, 384], F32)
            bo_sb = sb.tile([128, 4], F32)
            qkvt = sb.tile([128, 3, S], F32R)         # fb0=Q, fb1=K, fb2=V (h0|h1)
            vnat = sb.tile([128, NB, 130], F32R)      # [v0|1|v1|1] per key block
            valst = sb.tile([128, S], F32R)           # normalized attn @ V, d-major
            outt_sb = sb.tile([128, 4, S], F32)
            ident = sb.tile([128, 128], F32R)

            # weights/constants on the ACT HWDGE ring, xt on the SP ring;
            # both split by k-chunk so the first qkv group starts early
            for kc in range(4):
                nc.scalar.dma_start(out=wq_sb[:, kc, :], in_=wq[:, kc, :])
                nc.sync.dma_start(
                    out=xt_sb[:, kc, 0, :],
                    in_=xt[0, kc * 128:(kc + 1) * 128, :],
                )
            for cc in range(1, 4):
                nc.sync.dma_start(
                    out=xt_sb[:, :, cc, :],
                    in_=xt[cc].rearrange("(kc p) q -> p kc q", p=128),
                )
            nc.scalar.dma_start(out=km_sb, in_=km[:, :])
            nc.scalar.dma_start(out=tm2_sb, in_=tm[:, :])
            nc.sync.dma_start(out=ident, in_=idin[:, :])
            nc.sync.dma_start(out=wo_sb, in_=wo[:, :])
            nc.sync.dma_start(out=bo_sb, in_=bo4[:, :])

            # ones columns for the AV denominator rows; sourced from the
            # all-ones center block of the trimask (memset cannot write f32r)
            nc.vector.tensor_copy(vnat[:, :, 64:65], tm2_sb[:, 128:144])
            nc.vector.tensor_copy(vnat[:, :, 129:130], tm2_sb[:, 144:160])
            ones64 = sb.tile([1, 64], F32R)
            nc.vector.tensor_copy(ones64, tm2_sb[0:1, 128:192])


            # ---- qkv projection (transposed): qkvT = Wg^T @ x[b]^T ----
            for cc in range(4):
                for fb in range(3):
                    ps = ps_qkv.tile([128, 512], F32, tag="qkv")
                    for kc in range(4):
                        nc.tensor.matmul(
                            ps,
                            wq_sb[:, kc, fb * 128:(fb + 1) * 128],
                            xt_sb[:, kc, cc, :],
                            start=(kc == 0),
                            stop=(kc == 3),
                        )
                    nc.scalar.activation(
                        out=qkvt[:, fb, cc * 512:(cc + 1) * 512],
                        in_=ps,
                        func=mybir.ActivationFunctionType.Identity,
                    )

            # ---- V^T -> V natural [key, hd], with ones columns at 64/129 ----
            # 4 key blocks transposed into one PSUM bank, then one strided copy
            for kb0 in range(0, NB, 4):
                pst = ps_misc.tile([128, 4, 128], F32R, tag="misc", name="pst")
                for j in range(4):
                    kb = kb0 + j
                    nc.tensor.transpose(
                        pst[:, j, :], qkvt[:, 2, kb * 128:(kb + 1) * 128], ident
                    )
                nc.vector.tensor_copy(
                    vnat[:, kb0:kb0 + 4, :]
                    .rearrange("p k (g c) -> p k g c", c=65)[:, :, :, 0:64],
                    pst.rearrange("p k (g c) -> p k g c", c=64),
                )
            # ---- attention: heads interleaved per key block ----
            p_sb = [sb.tile([128, NB, 384], F32R, name=f"p{h}") for h in range(2)]

            def scores_block(h, kb):
                hp = 64 * h
                ws, we = max(0, kb - 1), min(NB - 1, kb + 1)
                nq = (we - ws + 1) * 128
                moff = (1 - (kb - ws)) * 128
                st = ps_st.tile([128, 384], F32, tag="st", name="st")
                nc.tensor.matmul(
                    st[:, :nq],
                    qkvt[hp:hp + 64, 1, kb * 128:(kb + 1) * 128],
                    qkvt[hp:hp + 64, 0, ws * 128:(we + 1) * 128],
                    start=True,
                    stop=True,
                )
                nc.scalar.activation(
                    out=p_sb[h][:, kb, 0:nq],
                    in_=st[:, :nq],
                    func=EXPF,
                    bias=km_sb[:, kb:kb + 1],
                    scale=0.125,
                )
                eng = nc.gpsimd if kb % 3 == 2 else nc.vector
                eng.tensor_mul(
                    p_sb[h][:, kb, 0:nq],
                    p_sb[h][:, kb, 0:nq],
                    tm2_sb[:, moff:moff + nq],
                )

            def av_chunk(h, c):
                hp = 64 * h
                kbs = list(range(max(0, 4 * c - 1), min(NB - 1, 4 * c + 4) + 1))
                ot = ps_ot.tile([65, 512], F32, tag="ot", name="ot")
                for i, kb in enumerate(kbs):
                    ws, we = max(0, kb - 1), min(NB - 1, kb + 1)
                    qs, qe = max(ws, 4 * c), min(we, 4 * c + 3)
                    nc.tensor.matmul(
                        ot[:, (qs - 4 * c) * 128:(qe + 1 - 4 * c) * 128],
                        vnat[:, kb, 65 * h:65 * h + 65],
                        p_sb[h][:, kb, (qs - ws) * 128:(qe + 1 - ws) * 128],
                        start=(i == 0),
                        stop=(i == len(kbs) - 1),
                        skip_group_check=True,
                    )
                rc = small.tile([1, 512], F32R, tag="rc", name="rc")
                with nc.allow_low_precision("f32r softmax denom recip"):
                    nc.vector.reciprocal(rc, ot[64:65, :])
                nc.scalar.activation(out=valst[hp:hp + 64, c * 512:(c + 1) * 512], in_=ot[0:64, :], func=mybir.ActivationFunctionType.Identity)
                rbp = ps_misc.tile([64, 512], F32, tag="misc", name="rbp")
                nc.tensor.matmul(rbp, ones64, rc, start=True, stop=True)
                nc.vector.tensor_mul(
                    valst[hp:hp + 64, c * 512:(c + 1) * 512],
                    valst[hp:hp + 64, c * 512:(c + 1) * 512],
                    rbp,
                )

            def oproj_chunk(c):
                for fo in range(4):
                    po = (ps_misc if fo % 2 == 0 else ps_ot).tile(
                        [128, 512], F32, tag="misc" if fo % 2 == 0 else "ot",
                        name="po")
                    nc.tensor.matmul(
                        po,
                        wo_sb[:, fo * 128:(fo + 1) * 128],
                        valst[:, c * 512:(c + 1) * 512],
                        start=True,
                        stop=True,
                    )
                    if fo % 2 == 0:
                        nc.scalar.activation(
                            out=outt_sb[:, fo, c * 512:(c + 1) * 512],
                            in_=po,
                            func=mybir.ActivationFunctionType.Identity,
                            bias=bo_sb[:, fo:fo + 1],
                        )
                    else:
                        nc.vector.tensor_scalar_add(
                            out=outt_sb[:, fo, c * 512:(c + 1) * 512],
                            in0=po,
                            scalar1=bo_sb[:, fo:fo + 1],
                        )
                    nc.sync.dma_start(
                        out=outt[fo * 128:(fo + 1) * 128, c * 512:(c + 1) * 512],
                        in_=outt_sb[:, fo, c * 512:(c + 1) * 512],
                    )

            for kb in range(NB):
                for h in range(2):
                    scores_block(h, kb)
                # chunk c's AV window ends at kb = 4c+4 (or NB-1 for the last)
                if kb >= 4 and kb % 4 == 0:
                    c = kb // 4 - 1
                    for h in range(2):
                        av_chunk(h, c)
                    oproj_chunk(c)
            for h in range(2):
                av_chunk(h, NCHUNK - 1)
            oproj_chunk(NCHUNK - 1)

    nc.finalize()
    return nc


def _numpy_reference(x, padding_mask, Wqkv, bqkv, Wo, bo):
    """Fallback for input regimes the device path does not cover."""
    b, s, _ = x.shape
    qkv = x @ Wqkv + bqkv
    qkv = qkv.reshape(b, s, H, 3 * HD).transpose(0, 2, 1, 3)
    q, k, v = np.split(qkv, 3, axis=-1)
    scores = np.einsum("bhqd,bhkd->bhqk", q, k)
    idx = np.arange(s)
    band = np.abs(idx[:, None] - idx[None, :]) <= 128
    pm = padding_mask != 0
    valid = band[None, None] & pm[:, None, None, :] & pm[:, None, :, None]
    scores = np.where(valid, scores, -np.inf) / np.sqrt(HD)
    scores = scores - scores.max(axis=-1, keepdims=True)
    with np.errstate(invalid="ignore", over="ignore"):
        e = np.exp(scores)
        attn = e / e.sum(axis=-1, keepdims=True)
    attn = np.nan_to_num(attn, nan=0.0)
    vals = np.einsum("bhqk,bhkd->bhqd", attn, v)
    vals = vals.transpose(0, 2, 1, 3).reshape(b, s, E)
    return (vals @ Wo + bo).astype(np.float32)


def kernel(x, padding_mask, Wqkv, bqkv, Wo, bo):
    global LAST_RESULTS
    x = np.ascontiguousarray(np.asarray(x, np.float32))
    Wqkv = np.asarray(Wqkv, np.float32)
    bqkv = np.asarray(bqkv, np.float32)
    Wo = np.asarray(Wo, np.float32)
    bo = np.asarray(bo, np.float32)
    pm = np.asarray(padding_mask)

    if np.any(bqkv != 0):
        # qkv bias is identically zero in the target problem; the device
        # program folds no qkv bias, so fall back rather than be wrong.
        return _numpy_reference(x, pm, Wqkv, bqkv, Wo, bo)

    if "nc" not in _CACHE:
        _CACHE["nc"] = _build_nc()
    nc = _CACHE["nc"]

    # trimask [key p, 384]: window cols = [qb-1 | qb | qb+1] relative blocks
    j = np.arange(128)[:, None]
    i = np.arange(128)[None, :]
    tm = np.concatenate(
        [(j <= i), np.ones((128, 128), bool), (j >= i)], axis=1
    ).astype(np.float32)

    in_maps = []
    for core in range(8):
        b, g = divmod(core, 4)
        # feature permutation for this head group: [q0|q1|k0|k1|v0|v1]
        h0, h1 = 2 * g, 2 * g + 1
        cols = []
        for kind in range(3):  # q, k, v
            for h in (h0, h1):
                base = h * 3 * HD + kind * HD
                cols.extend(range(base, base + HD))
        wq_g = Wqkv[:, cols]                                  # [512, 384]
        xt_b = np.ascontiguousarray(x[b].T)                   # [512, 2048]
        xt_cc = np.stack([xt_b[:, cc * 512:(cc + 1) * 512] for cc in range(4)])
        km = np.where(pm[b] != 0, 0.0, -1e5).astype(np.float32)
        in_maps.append({
            "xt": np.ascontiguousarray(xt_cc, dtype=np.float32),
            "wq": np.ascontiguousarray(
                wq_g.reshape(4, 128, 384).transpose(1, 0, 2), dtype=np.float32),
            "wo": np.ascontiguousarray(
                Wo[g * 128:(g + 1) * 128, :], dtype=np.float32),
            "km": np.ascontiguousarray(km.reshape(NB, 128).T, dtype=np.float32),
            "tm": tm,
            "bo4": np.ascontiguousarray(
                (bo / 4.0).reshape(4, 128).T, dtype=np.float32),
            "idin": np.eye(128, dtype=np.float32),
        })

    try:
        LAST_RESULTS = run_bass_kernel_spmd(nc, in_maps, core_ids=list(range(8)))
    except Exception:
        # transient device faults (e.g. NRT_EXEC_UNIT_UNRECOVERABLE) have been
        # observed to clear on the next attempt; retry once before giving up
        LAST_RESULTS = run_bass_kernel_spmd(nc, in_maps, core_ids=list(range(8)))
    res = LAST_RESULTS.results

    out = np.zeros((B, S, E), np.float32)
    for core in range(8):
        b = core // 4
        out[b] += res[core]["outt"].T
    return out



# revision 7
# speedup vs baseline: 1.0344x; 1.0344x over previous
"""Banded (sliding-window) multi-head attention on 8 Trainium2 NeuronCores.

Problem: B=2, S=2048, D=512, H=8 heads (hd=64), window=256 (|i-j| <= 128),
  qkv = x @ Wqkv + bqkv           -> per-head q,k,v
  scores = (q k^T masked to band) / 8 ; softmax ; out = (attn v) @ Wo + bo

Sharding: core = (batch b in {0,1}) x (head-group g in {0..3}); each core
computes 2 heads over the full sequence of one batch element plus the o_proj
partial product for its heads' embed slice. The host sums the 4 partials per
batch and adds bo once.

Device-side layout (bf16 matmul operands, fp32 PSUM accumulation):
  - qkv projection computed TRANSPOSED: qkvT[fo, s], features permuted to
    [q0|q1|k0|k1|v0|v1] so Q^T/K^T/V^T per head live at partition offsets
    {0,64}.
  - scores key-major per 128-key block kb vs the 1-3 query blocks in band:
    ST[key, query] in PSUM -> ACT: P = exp(ST/8 + kmask[key]) -> DVE: the two
    128-col edge blocks *= triangular band mask (strided single op).
  - V^T re-transposed to natural [key, hd] via PE (bf16 transposes), with a
    ones column at 64/129 so denominators fall out of the AV matmul.
  - AV is FLIPPED vs v1: per query block qb, out[128q, 65] = sum_kb
    P[k, qb]^T @ [V|1]: 65-col matmuls (cheap in the col-based PE cost
    model), and the softmax denominator lands in PSUM as a *column*
    [128q, 1] -> strided reciprocal over 4 query blocks costs ~130ns,
    and no broadcast matmul is needed: normalization is a per-partition
    tensor_scalar multiply in SBUF bf16 (4x DVE mode).
  - valst_nat[q, d] is re-transposed per (h, qb) on PE into a per-chunk
    PSUM tile -> one copy -> d-major valstT for o_proj.
  - o_proj: outT[fo, s] partial = Wo_g^T @ valstT, written bf16 (host sums
    partials in fp32 and adds bo).
"""

import numpy as np
import ml_dtypes

import concourse.bass as bass  # noqa: F401
import concourse.mybir as mybir
import concourse.tile as tile
from concourse import bacc
from concourse.bass_utils import run_bass_kernel_spmd

B, S, DIN, E = 2, 2048, 512, 512
H, HD = 8, 64
NB = S // 128      # 16 key/query blocks of 128
NCHUNK = S // 512  # 4 query chunks of 512
F32 = mybir.dt.float32
F32R = mybir.dt.float32r
BF16 = mybir.dt.bfloat16
EXPF = mybir.ActivationFunctionType.Exp
IDENT = mybir.ActivationFunctionType.Identity
BF = ml_dtypes.bfloat16

_CACHE = {}
LAST_RESULTS = None  # BassKernelResults of the most recent run (for test.py)


def _build_nc():
    nc = bacc.Bacc(None, target_bir_lowering=False, debug=False)

    xt = nc.dram_tensor("xt", [4, DIN, 512], BF16, kind="ExternalInput")
    wq = nc.dram_tensor("wq", [128, 4, 384], BF16, kind="ExternalInput")
    wo = nc.dram_tensor("wo", [128, E], BF16, kind="ExternalInput")
    km = nc.dram_tensor("km", [128, NB], F32, kind="ExternalInput")
    tm = nc.dram_tensor("tm", [128, 256], BF16, kind="ExternalInput")
    idin = nc.dram_tensor("idin", [128, 128], BF16, kind="ExternalInput")
    outt = nc.dram_tensor("outt", [E, S], BF16, kind="ExternalOutput")

    with tile.TileContext(nc) as tc:
        with (
            tc.tile_pool(name="sb", bufs=1) as sb,
            tc.tile_pool(name="ps_qkv", bufs=2, space="PSUM") as ps_qkv,
            tc.tile_pool(name="ps_st", bufs=2, space="PSUM") as ps_st,
            tc.tile_pool(name="ps_av", bufs=2, space="PSUM") as ps_av,
            tc.tile_pool(name="ps_t", bufs=2, space="PSUM") as ps_t,
            tc.tile_pool(name="small", bufs=4) as small,
        ):
            xt_sb = sb.tile([128, 4, 4, 512], BF16)   # [p, kchunk, qchunk, q]
            wq_sb = sb.tile([128, 4, 384], BF16)      # [p, kchunk, fo]
            wo_sb = sb.tile([128, E], BF16)
            km_sb = sb.tile([128, NB], F32)
            tmE_sb = sb.tile([128, 2, 128], BF16)     # [lower | upper] edges
            qkvt = sb.tile([128, 3, S], BF16)         # fb0=Q, fb1=K, fb2=V
            vnat = sb.tile([128, NB, 130], BF16)      # [v0|1|v1|1] per key blk
            vln = sb.tile([128, 2, 4, 64], BF16)      # valst_nat per (h, qb)
            rc_sb = sb.tile([128, 2, 4], F32)         # 1/denom per (h, qb)
            valstT = sb.tile([128, S], BF16)          # d-major normalized AV
            outt_sb = sb.tile([128, 4, S], BF16)
            ident = sb.tile([128, 128], BF16)

            # weights/constants on the ACT HWDGE ring, xt on the SP ring;
            # both split by k-chunk so the first qkv group starts early
            for kc in range(4):
                nc.scalar.dma_start(out=wq_sb[:, kc, :], in_=wq[:, kc, :])
                nc.sync.dma_start(
                    out=xt_sb[:, kc, 0, :],
                    in_=xt[0, kc * 128:(kc + 1) * 128, :],
                )
            for cc in range(1, 4):
                nc.sync.dma_start(
                    out=xt_sb[:, :, cc, :],
                    in_=xt[cc].rearrange("(kc p) q -> p kc q", p=128),
                )
            nc.scalar.dma_start(out=km_sb, in_=km[:, :])
            nc.scalar.dma_start(
                out=tmE_sb, in_=tm.rearrange("p (a b) -> p a b", b=128))
            nc.sync.dma_start(out=ident, in_=idin[:, :])
            nc.sync.dma_start(out=wo_sb, in_=wo[:, :])

            # ones columns for the AV denominator
            nc.gpsimd.memset(vnat[:, :, 64:65], 1.0)
            nc.gpsimd.memset(vnat[:, :, 129:130], 1.0)

            # ---- qkv projection (transposed): qkvT = Wg^T @ x[b]^T ----
            for cc in range(4):
                for fb in range(3):
                    ps = ps_qkv.tile([128, 512], F32, tag="qkv")
                    for kc in range(4):
                        nc.tensor.matmul(
                            ps,
                            wq_sb[:, kc, fb * 128:(fb + 1) * 128],
                            xt_sb[:, kc, cc, :],
                            start=(kc == 0),
                            stop=(kc == 3),
                        )
                    dst = qkvt[:, fb, cc * 512:(cc + 1) * 512]
                    if (cc, fb) in ((0, 0), (2, 0)):
                        nc.scalar.activation(out=dst, in_=ps, func=IDENT)
                    else:
                        nc.vector.tensor_copy(dst, ps)

            # ---- V^T -> V natural [key, hd] with ones cols at 64/129 ----
            for kb0 in range(0, NB, 4):
                pst = ps_t.tile([128, 4, 128], BF16, tag="t", name="pst")
                for j in range(4):
                    kb = kb0 + j
                    nc.tensor.transpose(
                        pst[:, j, :], qkvt[:, 2, kb * 128:(kb + 1) * 128],
                        ident,
                    )
                nc.vector.tensor_copy(
                    vnat[:, kb0:kb0 + 4, :]
                    .rearrange("p k (g c) -> p k g c", c=65)[:, :, :, 0:64],
                    pst.rearrange("p k (g c) -> p k g c", c=64),
                )

            # ---- attention ----
            p_sb = [sb.tile([128, NB, 384], BF16, name=f"p{h}")
                    for h in range(2)]

            def scores_block(h, kb):
                hp = 64 * h
                ws, we = max(0, kb - 1), min(NB - 1, kb + 1)
                nq = (we - ws + 1) * 128
                st = ps_st.tile([128, 384], F32, tag="st", name="st")
                nc.tensor.matmul(
                    st[:, :nq],
                    qkvt[hp:hp + 64, 1, kb * 128:(kb + 1) * 128],
                    qkvt[hp:hp + 64, 0, ws * 128:(we + 1) * 128],
                    start=True,
                    stop=True,
                )
                nc.scalar.activation(
                    out=p_sb[h][:, kb, 0:nq],
                    in_=st[:, :nq],
                    func=EXPF,
                    bias=km_sb[:, kb:kb + 1],
                    scale=0.125,
                )
                # band-mask only the edge blocks (lower on the kb-1 block,
                # upper on the kb+1 block); the center block is all-valid.
                # Pool takes a share (SBUF-only op) to unload DVE.
                eng = nc.gpsimd if (2 * kb + h) % 8 < 3 else nc.vector
                pv = p_sb[h][:, kb, :].rearrange("p (a b) -> p a b", b=128)
                if kb == 0:
                    eng.tensor_mul(
                        pv[:, 1, :], pv[:, 1, :], tmE_sb[:, 1, :])
                elif kb == NB - 1:
                    eng.tensor_mul(
                        pv[:, 0, :], pv[:, 0, :], tmE_sb[:, 0, :])
                else:
                    eng.tensor_mul(
                        pv[:, 0::2, :], pv[:, 0::2, :], tmE_sb)

            def av_chunk(h, c):
                # flipped AV: per query block, out[128q, 65] accumulates
                # P[k, qb]^T @ [V|1] over the 2-3 contributing key blocks
                av = ps_av.tile([128, 4, 65], F32, tag="av", name="av")
                for j in range(4):
                    qb = 4 * c + j
                    kbs = [kb for kb in (qb - 1, qb, qb + 1)
                           if 0 <= kb <= NB - 1]
                    for i, kb in enumerate(kbs):
                        ws = max(0, kb - 1)
                        nc.tensor.matmul(
                            av[:, j, :],
                            p_sb[h][:, kb, (qb - ws) * 128:(qb - ws + 1) * 128],
                            vnat[:, kb, 65 * h:65 * h + 65],
                            start=(i == 0),
                            stop=(i == len(kbs) - 1),
                        )
                with nc.allow_low_precision("f32r softmax denom recip"):
                    nc.vector.reciprocal(rc_sb[:, h, :], av[:, :, 64])
                nc.vector.tensor_copy(
                    vln[:, h, :, :], av[:, :, 0:64])
                for j in range(4):
                    nc.gpsimd.tensor_scalar_mul(
                        out=vln[:, h, j, :],
                        in0=vln[:, h, j, :],
                        scalar1=rc_sb[:, h, j:j + 1],
                    )

            def oproj_chunk(c):
                # transpose normalized [q, d] -> d-major [2h*64, 512q]
                vt = ps_t.tile([128, 512], BF16, tag="t", name="vt")
                for h in range(2):
                    for j in range(4):
                        nc.tensor.transpose(
                            vt[64 * h:64 * h + 64, j * 128:(j + 1) * 128],
                            vln[:, h, j, :],
                            ident,
                        )
                nc.vector.tensor_copy(
                    valstT[:, c * 512:(c + 1) * 512], vt)
                for fo in range(4):
                    po = ps_qkv.tile([128, 512], F32, tag="qkv", name="po")
                    nc.tensor.matmul(
                        po,
                        wo_sb[:, fo * 128:(fo + 1) * 128],
                        valstT[:, c * 512:(c + 1) * 512],
                        start=True,
                        stop=True,
                    )
                    dst = outt_sb[:, fo, c * 512:(c + 1) * 512]
                    if fo % 2 == 0:
                        nc.scalar.activation(out=dst, in_=po, func=IDENT)
                    else:
                        nc.vector.tensor_copy(dst, po)
                    nc.sync.dma_start(
                        out=outt[fo * 128:(fo + 1) * 128,
                                 c * 512:(c + 1) * 512],
                        in_=dst,
                    )

            for kb in range(NB):
                for h in range(2):
                    scores_block(h, kb)
                # chunk c's AV window ends at kb = 4c+4 (or NB-1 at the end)
                if kb >= 4 and kb % 4 == 0:
                    c = kb // 4 - 1
                    for h in range(2):
                        av_chunk(h, c)
                    oproj_chunk(c)
            for h in range(2):
                av_chunk(h, NCHUNK - 1)
            oproj_chunk(NCHUNK - 1)

    nc.finalize()
    return nc


def _numpy_reference(x, padding_mask, Wqkv, bqkv, Wo, bo):
    """Fallback for input regimes the device path does not cover."""
    b, s, _ = x.shape
    qkv = x @ Wqkv + bqkv
    qkv = qkv.reshape(b, s, H, 3 * HD).transpose(0, 2, 1, 3)
    q, k, v = np.split(qkv, 3, axis=-1)
    scores = np.einsum("bhqd,bhkd->bhqk", q, k)
    idx = np.arange(s)
    band = np.abs(idx[:, None] - idx[None, :]) <= 128
    pm = padding_mask != 0
    valid = band[None, None] & pm[:, None, None, :] & pm[:, None, :, None]
    scores = np.where(valid, scores, -np.inf) / np.sqrt(HD)
    scores = scores - scores.max(axis=-1, keepdims=True)
    with np.errstate(invalid="ignore", over="ignore"):
        e = np.exp(scores)
        attn = e / e.sum(axis=-1, keepdims=True)
    attn = np.nan_to_num(attn, nan=0.0)
    vals = np.einsum("bhqk,bhkd->bhqd", attn, v)
    vals = vals.transpose(0, 2, 1, 3).reshape(b, s, E)
    return (vals @ Wo + bo).astype(np.float32)


def kernel(x, padding_mask, Wqkv, bqkv, Wo, bo):
    global LAST_RESULTS
    x = np.ascontiguousarray(np.asarray(x, np.float32))
    Wqkv = np.asarray(Wqkv, np.float32)
    bqkv = np.asarray(bqkv, np.float32)
    Wo = np.asarray(Wo, np.float32)
    bo = np.asarray(bo, np.float32)
    pm = np.asarray(padding_mask)

    if np.any(bqkv != 0):
        # qkv bias is identically zero in the target problem; the device
        # program folds no qkv bias, so fall back rather than be wrong.
        return _numpy_reference(x, pm, Wqkv, bqkv, Wo, bo)

    if "nc" not in _CACHE:
        _CACHE["nc"] = _build_nc()
    nc = _CACHE["nc"]

    # band mask edge blocks [key p, {lower, upper}]
    j = np.arange(128)[:, None]
    i = np.arange(128)[None, :]
    tm = np.concatenate([(j <= i), (j >= i)], axis=1).astype(BF)

    in_maps = []
    for core in range(8):
        b, g = divmod(core, 4)
        # feature permutation for this head group: [q0|q1|k0|k1|v0|v1]
        h0, h1 = 2 * g, 2 * g + 1
        cols = []
        for kind in range(3):  # q, k, v
            for h in (h0, h1):
                base = h * 3 * HD + kind * HD
                cols.extend(range(base, base + HD))
        wq_g = Wqkv[:, cols]                                  # [512, 384]
        xt_b = np.ascontiguousarray(x[b].T)                   # [512, 2048]
        xt_cc = np.stack([xt_b[:, cc * 512:(cc + 1) * 512] for cc in range(4)])
        km = np.where(pm[b] != 0, 0.0, -1e5).astype(np.float32)
        in_maps.append({
            "xt": np.ascontiguousarray(xt_cc).astype(BF),
            "wq": np.ascontiguousarray(
                wq_g.reshape(4, 128, 384).transpose(1, 0, 2)).astype(BF),
            "wo": np.ascontiguousarray(Wo[g * 128:(g + 1) * 128, :]).astype(BF),
            "km": np.ascontiguousarray(km.reshape(NB, 128).T,
                                       dtype=np.float32),
            "tm": tm,
            "idin": np.eye(128, dtype=BF),
        })

    try:
        LAST_RESULTS = run_bass_kernel_spmd(nc, in_maps, core_ids=list(range(8)))
    except Exception:
        # transient device faults (e.g. NRT_EXEC_UNIT_UNRECOVERABLE) have been
        # observed to clear on the next attempt; retry once before giving up
        LAST_RESULTS = run_bass_kernel_spmd(nc, in_maps, core_ids=list(range(8)))
    res = LAST_RESULTS.results

    out = np.zeros((B, S, E), np.float32)
    for core in range(8):
        b = core // 4
        out[b] += np.asarray(res[core]["outt"], np.float32).T
    out += bo
    return out
